# revision 16
# baseline (speedup 1.0000x reference)
"""Real spherical harmonics Y_lm (l<=8) on 8 TRN2 NeuronCores.

Data-parallel over the 1M points; per core 125k points padded to
128*977. All device compute and the output are fp16 (tolerance is
2e-2; fp16 end-to-end lands ~2e-3): DVE tensor_tensor runs at 2x and
tensor_scalar at 4x with packed 2-byte operands, while
scalar_tensor_tensor gets no speedup — so the three-term Legendre
recurrences are rescaled per (l,m) chain (P' = g*P~) to force the
P(l-2) coefficient to exactly -1, and all remaining per-(l,m) scalar
multiplies are hoisted into ACT Copy-scale prescales or DVE 4x
tensor_scalar ops, leaving the DVE inner loops pure fp16 TT.

The radial chains advance in l-lockstep over a level-major P' array
(slot (l,m) at l(l-1)/2 + m-1), so each level's recurrence runs as two
wide TTs across all m chains at once, and each level's 2l output
columns (diagonal included) are emitted by two wide TTs against
contiguous sin/cos blocks. The output tile is column-major, columns
grouped [m0 | per level l: sin m=1..l, cos m=1..l]; HBM mirrors SBUF
so each DMA stripe is one contiguous multi-KB run per partition,
striped 4x per chunk and fired as levels complete. The host undoes
chunk framing and the column permutation, applies per-column 1/g
scales (and odd-m sign flips from the sqrt-based diagonal seed), and
casts to f32.
"""

import math
import sys

sys.path.insert(0, "/opt/trn_rl_repo")

import numpy as np

import concourse.bass as bass
import concourse.mybir as mybir
from concourse.ap import AP
from concourse.tile import TileContext
from concourse.bass_utils import run_bass_kernel_spmd

F32 = mybir.dt.float32
F16 = mybir.dt.float16
AF = mybir.ActivationFunctionType
OP = mybir.AluOpType

N_TOTAL = 1_000_000
NCORES = 8
PER = N_TOTAL // NCORES      # 125000 real points per core
P = 128                      # SBUF partitions
LPP = 977                    # points per partition (128*977 = 125056)
PADN = P * LPP
LMAX = 8
NCOL = (LMAX + 1) ** 2       # 81

PI_LO = float(np.nextafter(np.float32(math.pi), np.float32(0.0)))


def _lbase(l):
    """device column base of the level-l group (sin m=1..l, cos m=1..l)."""
    return 9 + l * (l - 1)


def _devcol(l, m_signed):
    if m_signed == 0:
        return l
    m = abs(m_signed)
    return _lbase(l) + (0 if m_signed < 0 else l) + (m - 1)


def _ctil():
    c = {}
    for l in range(LMAX + 1):
        c[(l, 0)] = math.sqrt((2 * l + 1) / (4 * math.pi))
        for m in range(1, l + 1):
            c[(l, m)] = -((-1.0) ** m) * math.sqrt(2.0) * math.sqrt(
                (2 * l + 1) / (4 * math.pi)
                * math.factorial(l - m) / math.factorial(l + m)
            )
    return c


def _coeffs():
    """xa[(l,m)]: scalar on x in  P'(l) = (xa*x) . P'(l-1) - P'(l-2);
    g[(l,m)]: P'(l,m) = g * P~(l,m) (seeds g=1)."""
    C = _ctil()
    xa, g = {}, {}
    for m in range(0, LMAX):
        g[(m, m)] = 1.0
        g[(m + 1, m)] = 1.0
        for l in range(m + 2, LMAX + 1):
            alpha = (2 * l - 1) / (l - m)
            beta = -(l + m - 1) / (l - m)
            a2 = alpha * C[(l, m)] / C[(l - 1, m)]
            b2 = beta * C[(l, m)] / C[(l - 2, m)]
            g[(l, m)] = -g[(l - 2, m)] / b2
            xa[(l, m)] = a2 * g[(l, m)] / g[(l - 1, m)]
    g[(8, 8)] = 1.0
    return xa, g, C


def _host_maps():
    """(order, scale): full_out[:, j_true] = dev[:, order[j_true]] * scale[j_true]."""
    _, g, _ = _coeffs()
    order = np.zeros(NCOL, np.int64)
    scale = np.ones(NCOL, np.float32)
    for l in range(0, 9):
        for ms in range(-l, l + 1):
            j = l * l + l + ms
            order[j] = _devcol(l, ms)
            m = abs(ms)
            flip = -1.0 if (m % 2) else 1.0
            scale[j] = (flip if m else 1.0) / g[(l, m)]
    return order, scale


def _wpair(a, stride_elems):
    """[P, f] AP -> [P, 2, f] where the second copy sits +stride_elems."""
    d = a.ap
    assert len(d) == 2
    return AP(a.tensor, a.offset, [list(d[0]), [stride_elems, 2], list(d[1])])


def build_nc(fds):
    assert sum(fds) == LPP
    fdmax = max(fds)
    xa, g, C = _coeffs()
    C00, C10, C11 = C[(0, 0)], C[(1, 0)], C[(1, 1)]

    nc = bass.Bass()
    ct = nc.declare_dram_parameter("cos_theta", [PADN], F32, isOutput=False)
    ph = nc.declare_dram_parameter("phi", [PADN], F32, isOutput=False)
    out = nc.declare_dram_parameter("out", [PADN * NCOL], F16, isOutput=True)

    ctv = ct[:].rearrange("(p f) -> p f", p=P)
    phv = ph[:].rearrange("(p f) -> p f", p=P)
    outv = out[:].rearrange("(p q) -> p q", p=P)

    # work-tile f16 slice indices
    S_XH = 0
    S_S2 = 1
    S_2C = 2      # twoC1 doubled (2 slices)
    S_U = 4       # u pair (2 slices)
    S_SIN = 6     # sin m=1..8 (8 slices)
    S_COS = 14    # cos m=1..8 (8 slices)
    S_XS = 22     # XAS prescale batch (7 slices)
    S_TM = 29     # T batch scratch (6 slices)
    S_T0 = 35     # m0 chain scratch
    S_MS = 36     # ACT prescale scratch: m0
    S_DS = 37     # ACT prescale scratch: diag
    S_FS = 38     # ACT prescale scratch: first-l
    S_P = 39      # P'(l,m) level-major: slot (l,m) = l(l-1)/2 + m-1 (36)
    NSL = S_P + 36  # 75

    def psl(l, m):
        return S_P + l * (l - 1) // 2 + (m - 1)

    # output DMA stripes: (device col range, fires after level).
    # Fine-grained so issue spreads across the compute window — stripes
    # share the 16 DMA queues, so late issue serializes into the tail.
    STRIPES = [
        (9, _lbase(3), 2),
        (_lbase(3), _lbase(5), 4),
        (_lbase(5), _lbase(6), 5),
        (_lbase(6), _lbase(7), 6),
        (_lbase(7), _lbase(8), 7),
        (_lbase(8), NCOL, 8),
    ]

    with TileContext(nc) as tc:
        with (
            tc.tile_pool(name="res", bufs=1) as res_pool,
            tc.tile_pool(name="work", bufs=2) as work_pool,
            tc.tile_pool(name="obuf", bufs=1) as o_pool,
        ):
            xt = res_pool.tile([P, LPP], F32)
            pt = res_pool.tile([P, LPP], F32)
            cbias = res_pool.tile([P, 3], F32)
            nc.gpsimd.memset(cbias[:, 0:1], -PI_LO)
            nc.gpsimd.memset(cbias[:, 1:2], -PI_LO / 2)
            nc.gpsimd.memset(cbias[:, 2:3], C11 * C11)
            bias_negpi = cbias[:, 0:1]
            bias_neghalfpi = cbias[:, 1:2]
            bias_c11sq = cbias[:, 2:3]
            # trig T'_0 = (0, -1) constant pair
            t0c = res_pool.tile([P, 2 * fdmax], F16)
            nc.gpsimd.memset(t0c[:, 0:fdmax], 0.0)
            nc.gpsimd.memset(t0c[:, fdmax:2 * fdmax], -1.0)

            off = 0
            for fd in fds:
                sl = slice(off, off + fd)
                ooff = off
                off += fd
                nc.sync.dma_start(out=pt[:, sl], in_=phv[:, sl])
                nc.sync.dma_start(out=xt[:, sl], in_=ctv[:, sl])
                x = xt[:, sl]
                f = pt[:, sl]

                w = work_pool.tile([P, NSL * fd], F16)
                x2f = work_pool.tile([P, fd], F32)
                bbf = work_pool.tile([P, fd], F32)
                b2f = work_pool.tile([P, fd], F32)

                def W(i):
                    return w[:, i * fd:(i + 1) * fd]

                def WB(i, k):
                    return w[:, i * fd:(i + k) * fd].rearrange(
                        "p (k f) -> p k f", k=k
                    )

                T0P = t0c[:, :].rearrange("p (k f) -> p k f", k=2)[:, :, 0:fd]

                def TRIGP(m):
                    # (sin_m, cos_m) pair: slices S_SIN+m-1, S_COS+m-1
                    return _wpair(W(S_SIN + m - 1), 8 * fd)

                xh, s2h = W(S_XH), W(S_S2)

                O = o_pool.tile([P, NCOL * fd], F16)
                O3 = O.rearrange("p (c f) -> p c f", c=NCOL)

                def ocol(j):
                    return O3[:, j, :]

                # ---- ACT: transcendental + affine seeds ----
                # phi-side first (trig seeds gate the DVE trig chain)
                nc.scalar.activation(
                    W(S_SIN), f, AF.Sin, bias=bias_negpi
                )  # -sin(phi)
                nc.scalar.activation(
                    bbf, f, AF.Sin, scale=0.5, bias=bias_neghalfpi
                )  # -cos(phi/2)
                nc.scalar.activation(b2f, bbf, AF.Square)
                nc.scalar.activation(W(S_2C), b2f, AF.Copy, scale=4.0, bias=-2.0)
                nc.scalar.activation(
                    W(S_2C + 1), b2f, AF.Copy, scale=4.0, bias=-2.0
                )
                nc.scalar.activation(
                    W(S_COS), b2f, AF.Copy, scale=-2.0, bias=1.0
                )  # -cos(phi)
                nc.scalar.activation(x2f, x, AF.Square)
                nc.scalar.activation(xh, x, AF.Copy)
                nc.scalar.activation(s2h, x2f, AF.Copy, scale=-1.0, bias=1.0)
                # +C11*s == -P~(1,1): odd-m sign fixed on host
                nc.scalar.activation(
                    W(psl(1, 1)), x2f, AF.Sqrt,
                    scale=-(C11 * C11), bias=bias_c11sq,
                )
                nc.scalar.activation(
                    W(psl(2, 2)), x2f, AF.Copy,
                    scale=-3.0 * C[(2, 2)], bias=3.0 * C[(2, 2)],
                )

                # ---- trig chain: T'_m = twoC1*T'_{m-1} - T'_{m-2} ----
                for m in range(2, 9):
                    prev2 = T0P if m == 2 else TRIGP(m - 2)
                    nc.vector.tensor_tensor(
                        WB(S_U, 2), WB(S_2C, 2), TRIGP(m - 1), OP.mult
                    )
                    nc.vector.tensor_tensor(
                        TRIGP(m), WB(S_U, 2), prev2, OP.subtract
                    )

                # ---- O constants + m=0 chain (device cols 0..8) ----
                nc.gpsimd.memset(ocol(0), C00)
                nc.vector.tensor_scalar(ocol(1), xh, C10, None, OP.mult)
                T0 = W(S_T0)
                MS = W(S_MS)
                nc.scalar.activation(MS, ocol(1), AF.Copy, scale=xa[(2, 0)])
                nc.vector.tensor_tensor(T0, MS, xh, OP.mult)
                nc.vector.tensor_scalar(ocol(2), T0, C00, None, OP.subtract)
                for l in range(3, 9):
                    nc.scalar.activation(
                        MS, ocol(l - 1), AF.Copy, scale=xa[(l, 0)]
                    )
                    nc.vector.tensor_tensor(T0, MS, xh, OP.mult)
                    nc.vector.tensor_tensor(
                        ocol(l), T0, ocol(l - 2), OP.subtract
                    )
                nc.sync.dma_start(
                    out=outv[:, ooff * NCOL:ooff * NCOL + 9 * fd],
                    in_=O[:, 0:9 * fd],
                )

                # ---- levels l=1..8: lockstep chains + block emits ----
                stripe_i = 0
                for l in range(1, 9):
                    if l >= 3:
                        Al = (2 * l - 1) * (2 * l - 3) * C[(l, l)] / C[(l - 2, l - 2)]
                        nc.scalar.activation(
                            W(S_DS), W(psl(l - 2, l - 2)), AF.Copy, scale=Al
                        )
                        nc.vector.tensor_tensor(
                            W(psl(l, l)), W(S_DS), s2h, OP.mult
                        )
                    if l >= 2:
                        Em = (2 * l - 1) * C[(l, l - 1)] / C[(l - 1, l - 1)]
                        nc.scalar.activation(
                            W(S_FS), W(psl(l - 1, l - 1)), AF.Copy, scale=Em
                        )
                        nc.vector.tensor_tensor(
                            W(psl(l, l - 1)), W(S_FS), xh, OP.mult
                        )
                    if l >= 3:
                        nm = l - 2
                        for m in range(1, l - 1):
                            nc.vector.tensor_scalar(
                                W(S_XS + m - 1), xh, xa[(l, m)], None, OP.mult
                            )
                        nc.vector.tensor_tensor(
                            WB(S_TM, nm), WB(S_XS, nm),
                            WB(psl(l - 1, 1), nm), OP.mult,
                        )
                        nc.vector.tensor_tensor(
                            WB(psl(l, 1), nm), WB(S_TM, nm),
                            WB(psl(l - 2, 1), nm), OP.subtract,
                        )
                    # emits: sin block then cos block (diagonal included)
                    cb = _lbase(l)
                    nc.vector.tensor_tensor(
                        O3[:, cb:cb + l, :], WB(psl(l, 1), l),
                        WB(S_SIN, l), OP.mult,
                    )
                    nc.vector.tensor_tensor(
                        O3[:, cb + l:cb + 2 * l, :], WB(psl(l, 1), l),
                        WB(S_COS, l), OP.mult,
                    )
                    while stripe_i < len(STRIPES) and STRIPES[stripe_i][2] == l:
                        c0, c1, _ = STRIPES[stripe_i]
                        qb = ooff * NCOL
                        nc.sync.dma_start(
                            out=outv[:, qb + c0 * fd:qb + c1 * fd],
                            in_=O[:, c0 * fd:c1 * fd],
                        )
                        stripe_i += 1
    _legalize_waits(nc)
    return nc


def _legalize_waits(nc):
    """TPB compute ISA structs encode a single sync-wait slot; Tile can
    emit 2+ waits on one instruction. Hoist extras onto NoOps."""
    f = nc.m.functions[0]
    for b in f.blocks:
        insts = b.instructions
        idx = 0
        while idx < len(insts):
            i = insts[idx]
            si = i.sync_info
            if si is not None and len(si.on_wait) > 1:
                waits = list(si.on_wait)
                for wextra in waits[:-1]:
                    nop = mybir.InstEventSemaphore(
                        name=nc.get_next_instruction_name(), ins=[], outs=[]
                    )
                    nop.engine = i.engine
                    nop.sync_info = mybir.SyncInfo(
                        on_wait=[wextra], on_update=[]
                    )
                    nc.register_instruction(nop)
                    insts.insert(idx, nop)
                    idx += 1
                si.on_wait = [waits[-1]]
            idx += 1


_NC_CACHE = None

# Smaller final chunk shrinks the exposed tail DMA.
FDS = [405, 405, 167]


def _get_nc():
    global _NC_CACHE
    if _NC_CACHE is None:
        _NC_CACHE = build_nc(FDS)
    return _NC_CACHE


def _run(cos_theta, phi, trace=False, **kw):
    cos_theta = np.ascontiguousarray(np.asarray(cos_theta), dtype=np.float32)
    phi = np.ascontiguousarray(np.asarray(phi), dtype=np.float32)
    assert cos_theta.shape == (N_TOTAL,) and phi.shape == (N_TOTAL,)
    in_maps = []
    for i in range(NCORES):
        c = np.zeros(PADN, np.float32)
        p_ = np.zeros(PADN, np.float32)
        c[:PER] = cos_theta[i * PER:(i + 1) * PER]
        p_[:PER] = phi[i * PER:(i + 1) * PER]
        in_maps.append({"cos_theta": c, "phi": p_})
    res = run_bass_kernel_spmd(
        _get_nc(), in_maps, core_ids=list(range(NCORES)), trace=trace, **kw
    )
    order, scale = _host_maps()
    outs = []
    for r in res.results:
        a = np.asarray(r["out"]).reshape(P, NCOL * LPP)
        parts, q = [], 0
        for fd in FDS:  # undo per-chunk framing -> [P, NCOL, LPP]
            parts.append(a[:, q:q + NCOL * fd].reshape(P, NCOL, fd))
            q += NCOL * fd
        dev = np.concatenate(parts, axis=2)
        dev = dev.transpose(0, 2, 1).reshape(PADN, NCOL)
        outs.append(dev[:PER, order].astype(np.float32) * scale[None, :])
    return np.concatenate(outs, axis=0), res


def kernel(cos_theta, phi):
    out, _ = _run(cos_theta, phi)
    return out


# revision 17
# speedup vs baseline: 1.2617x; 1.2617x over previous
"""Real spherical harmonics Y_lm (l<=8) on 8 TRN2 NeuronCores.

Data-parallel over the 1M points; per core 125k points padded to
128*977. All device compute and the output are fp16 (tolerance is
2e-2; fp16 end-to-end lands ~2e-3): DVE tensor_tensor runs at 2x and
tensor_scalar at 4x with packed 2-byte operands, while
scalar_tensor_tensor gets no speedup — so the three-term Legendre
recurrences are rescaled per (l,m) chain (P' = g*P~) to force the
P(l-2) coefficient to exactly -1, and all remaining per-(l,m) scalar
multiplies are hoisted into ACT Copy-scale prescales or DVE 4x
tensor_scalar ops, leaving the DVE inner loops pure fp16 TT.

The radial chains advance in l-lockstep over a level-major P' array
(slot (l,m) at l(l-1)/2 + m-1), so each level's recurrence runs as two
wide TTs across all m chains at once, and each level's 2l output
columns (diagonal included) are emitted by two wide TTs against
contiguous sin/cos blocks. The output tile is column-major, columns
grouped [m0 | per level l: sin m=1..l, cos m=1..l]; HBM mirrors SBUF
so each DMA stripe is one contiguous multi-KB run per partition,
striped 4x per chunk and fired as levels complete. The host undoes
chunk framing and the column permutation, applies per-column 1/g
scales (and odd-m sign flips from the sqrt-based diagonal seed), and
casts to f32.
"""

import math
import sys

sys.path.insert(0, "/opt/trn_rl_repo")

import numpy as np

import concourse.bass as bass
import concourse.mybir as mybir
from concourse.ap import AP
from concourse.tile import TileContext
from concourse.bass_utils import run_bass_kernel_spmd

F32 = mybir.dt.float32
F16 = mybir.dt.float16
AF = mybir.ActivationFunctionType
OP = mybir.AluOpType

N_TOTAL = 1_000_000
NCORES = 8
PER = N_TOTAL // NCORES      # 125000 real points per core
P = 128                      # SBUF partitions
LPP = 977                    # points per partition (128*977 = 125056)
PADN = P * LPP
LMAX = 8
NCOL = (LMAX + 1) ** 2       # 81

PI_LO = float(np.nextafter(np.float32(math.pi), np.float32(0.0)))


def _lbase(l):
    """device column base of the level-l group (sin m=1..l, cos m=1..l)."""
    return 9 + l * (l - 1)


def _devcol(l, m_signed):
    if m_signed == 0:
        return l
    m = abs(m_signed)
    return _lbase(l) + (0 if m_signed < 0 else l) + (m - 1)


def _ctil():
    c = {}
    for l in range(LMAX + 1):
        c[(l, 0)] = math.sqrt((2 * l + 1) / (4 * math.pi))
        for m in range(1, l + 1):
            c[(l, m)] = -((-1.0) ** m) * math.sqrt(2.0) * math.sqrt(
                (2 * l + 1) / (4 * math.pi)
                * math.factorial(l - m) / math.factorial(l + m)
            )
    return c


def _coeffs():
    """xa[(l,m)]: scalar on x in  P'(l) = (xa*x) . P'(l-1) - P'(l-2);
    g[(l,m)]: P'(l,m) = g * P~(l,m) (seeds g=1)."""
    C = _ctil()
    xa, g = {}, {}
    for m in range(0, LMAX):
        g[(m, m)] = 1.0
        g[(m + 1, m)] = 1.0
        for l in range(m + 2, LMAX + 1):
            alpha = (2 * l - 1) / (l - m)
            beta = -(l + m - 1) / (l - m)
            a2 = alpha * C[(l, m)] / C[(l - 1, m)]
            b2 = beta * C[(l, m)] / C[(l - 2, m)]
            g[(l, m)] = -g[(l - 2, m)] / b2
            xa[(l, m)] = a2 * g[(l, m)] / g[(l - 1, m)]
    g[(8, 8)] = 1.0
    return xa, g, C


def _host_maps():
    """(order, scale): full_out[:, j_true] = dev[:, order[j_true]] * scale[j_true]."""
    _, g, _ = _coeffs()
    order = np.zeros(NCOL, np.int64)
    scale = np.ones(NCOL, np.float32)
    for l in range(0, 9):
        for ms in range(-l, l + 1):
            j = l * l + l + ms
            order[j] = _devcol(l, ms)
            m = abs(ms)
            flip = -1.0 if (m % 2) else 1.0
            scale[j] = (flip if m else 1.0) / g[(l, m)]
    return order, scale


def _wpair(a, stride_elems):
    """[P, f] AP -> [P, 2, f] where the second copy sits +stride_elems."""
    d = a.ap
    assert len(d) == 2
    return AP(a.tensor, a.offset, [list(d[0]), [stride_elems, 2], list(d[1])])


def build_nc(fds):
    assert sum(fds) == LPP
    fdmax = max(fds)
    xa, g, C = _coeffs()
    C00, C10, C11 = C[(0, 0)], C[(1, 0)], C[(1, 1)]

    nc = bass.Bass()
    ct = nc.declare_dram_parameter("cos_theta", [PADN], F32, isOutput=False)
    ph = nc.declare_dram_parameter("phi", [PADN], F32, isOutput=False)
    out = nc.declare_dram_parameter("out", [PADN * NCOL], F16, isOutput=True)

    ctv = ct[:].rearrange("(p f) -> p f", p=P)
    phv = ph[:].rearrange("(p f) -> p f", p=P)
    outv = out[:].rearrange("(p q) -> p q", p=P)

    # work-tile f16 slice indices
    S_XH = 0
    S_S2 = 1
    S_2C = 2      # twoC1 doubled (2 slices)
    S_U = 4       # u pair (2 slices)
    S_SIN = 6     # sin m=1..8 (8 slices)
    S_COS = 14    # cos m=1..8 (8 slices)
    S_XS = 22     # XAS prescale batch (7 slices)
    S_TM = 29     # T batch scratch (6 slices)
    S_T0 = 35     # m0 chain scratch
    S_MS = 36     # ACT prescale scratch: m0
    S_DS = 37     # ACT prescale scratch: diag
    S_FS = 38     # ACT prescale scratch: first-l
    S_P = 39      # P'(l,m) level-major: slot (l,m) = l(l-1)/2 + m-1 (36)
    NSL = S_P + 36  # 75

    def psl(l, m):
        return S_P + l * (l - 1) // 2 + (m - 1)

    # output DMA stripes: (device col range, fires after level).
    # Fine-grained so issue spreads across the compute window — stripes
    # share the 16 DMA queues, so late issue serializes into the tail.
    STRIPES = [
        (9, _lbase(3), 2),
        (_lbase(3), _lbase(5), 4),
        (_lbase(5), _lbase(6), 5),
        (_lbase(6), _lbase(7), 6),
        (_lbase(7), _lbase(8), 7),
        (_lbase(8), NCOL, 8),
    ]

    with TileContext(nc) as tc:
        with (
            tc.tile_pool(name="res", bufs=1) as res_pool,
            tc.tile_pool(name="work", bufs=2) as work_pool,
            tc.tile_pool(name="obuf", bufs=1) as o_pool,
        ):
            xt = res_pool.tile([P, LPP], F32)
            pt = res_pool.tile([P, LPP], F32)
            cbias = res_pool.tile([P, 3], F32)
            nc.gpsimd.memset(cbias[:, 0:1], -PI_LO)
            nc.gpsimd.memset(cbias[:, 1:2], -PI_LO / 2)
            nc.gpsimd.memset(cbias[:, 2:3], C11 * C11)
            bias_negpi = cbias[:, 0:1]
            bias_neghalfpi = cbias[:, 1:2]
            bias_c11sq = cbias[:, 2:3]
            # trig T'_0 = (0, -1) constant pair
            t0c = res_pool.tile([P, 2 * fdmax], F16)
            nc.gpsimd.memset(t0c[:, 0:fdmax], 0.0)
            nc.gpsimd.memset(t0c[:, fdmax:2 * fdmax], -1.0)

            off = 0
            for fd in fds:
                sl = slice(off, off + fd)
                ooff = off
                off += fd
                nc.sync.dma_start(out=pt[:, sl], in_=phv[:, sl])
                nc.sync.dma_start(out=xt[:, sl], in_=ctv[:, sl])
                x = xt[:, sl]
                f = pt[:, sl]

                w = work_pool.tile([P, NSL * fd], F16)
                x2f = work_pool.tile([P, fd], F32)
                bbf = work_pool.tile([P, fd], F32)
                b2f = work_pool.tile([P, fd], F32)

                def W(i):
                    return w[:, i * fd:(i + 1) * fd]

                def WB(i, k):
                    return w[:, i * fd:(i + k) * fd].rearrange(
                        "p (k f) -> p k f", k=k
                    )

                T0P = t0c[:, :].rearrange("p (k f) -> p k f", k=2)[:, :, 0:fd]

                def TRIGP(m):
                    # (sin_m, cos_m) pair: slices S_SIN+m-1, S_COS+m-1
                    return _wpair(W(S_SIN + m - 1), 8 * fd)

                xh, s2h = W(S_XH), W(S_S2)

                O = o_pool.tile([P, NCOL * fd], F16)
                O3 = O.rearrange("p (c f) -> p c f", c=NCOL)

                def ocol(j):
                    return O3[:, j, :]

                # ---- ACT: transcendental + affine seeds ----
                # phi-side first (trig seeds gate the DVE trig chain)
                nc.scalar.activation(
                    W(S_SIN), f, AF.Sin, bias=bias_negpi
                )  # -sin(phi)
                nc.scalar.activation(
                    bbf, f, AF.Sin, scale=0.5, bias=bias_neghalfpi
                )  # -cos(phi/2)
                nc.scalar.activation(b2f, bbf, AF.Square)
                nc.scalar.activation(W(S_2C), b2f, AF.Copy, scale=4.0, bias=-2.0)
                nc.scalar.activation(
                    W(S_2C + 1), b2f, AF.Copy, scale=4.0, bias=-2.0
                )
                nc.scalar.activation(
                    W(S_COS), b2f, AF.Copy, scale=-2.0, bias=1.0
                )  # -cos(phi)
                nc.scalar.activation(x2f, x, AF.Square)
                nc.scalar.activation(xh, x, AF.Copy)
                nc.scalar.activation(s2h, x2f, AF.Copy, scale=-1.0, bias=1.0)
                # +C11*s == -P~(1,1): odd-m sign fixed on host
                nc.scalar.activation(
                    W(psl(1, 1)), x2f, AF.Sqrt,
                    scale=-(C11 * C11), bias=bias_c11sq,
                )
                nc.scalar.activation(
                    W(psl(2, 2)), x2f, AF.Copy,
                    scale=-3.0 * C[(2, 2)], bias=3.0 * C[(2, 2)],
                )

                # ---- trig chain: T'_m = twoC1*T'_{m-1} - T'_{m-2} ----
                for m in range(2, 9):
                    prev2 = T0P if m == 2 else TRIGP(m - 2)
                    nc.vector.tensor_tensor(
                        WB(S_U, 2), WB(S_2C, 2), TRIGP(m - 1), OP.mult
                    )
                    nc.vector.tensor_tensor(
                        TRIGP(m), WB(S_U, 2), prev2, OP.subtract
                    )

                # ---- O constants + m=0 chain (device cols 0..8) ----
                nc.gpsimd.memset(ocol(0), C00)
                nc.vector.tensor_scalar(ocol(1), xh, C10, None, OP.mult)
                T0 = W(S_T0)
                MS = W(S_MS)
                nc.scalar.activation(MS, ocol(1), AF.Copy, scale=xa[(2, 0)])
                nc.vector.tensor_tensor(T0, MS, xh, OP.mult)
                nc.vector.tensor_scalar(ocol(2), T0, C00, None, OP.subtract)
                for l in range(3, 9):
                    nc.scalar.activation(
                        MS, ocol(l - 1), AF.Copy, scale=xa[(l, 0)]
                    )
                    nc.vector.tensor_tensor(T0, MS, xh, OP.mult)
                    nc.vector.tensor_tensor(
                        ocol(l), T0, ocol(l - 2), OP.subtract
                    )
                nc.sync.dma_start(
                    out=outv[:, ooff * NCOL:ooff * NCOL + 9 * fd],
                    in_=O[:, 0:9 * fd],
                )

                # ---- levels l=1..8: lockstep chains + block emits ----
                stripe_i = 0
                for l in range(1, 9):
                    if l >= 3:
                        Al = (2 * l - 1) * (2 * l - 3) * C[(l, l)] / C[(l - 2, l - 2)]
                        nc.scalar.activation(
                            W(S_DS), W(psl(l - 2, l - 2)), AF.Copy, scale=Al
                        )
                        nc.vector.tensor_tensor(
                            W(psl(l, l)), W(S_DS), s2h, OP.mult
                        )
                    if l >= 2:
                        Em = (2 * l - 1) * C[(l, l - 1)] / C[(l - 1, l - 1)]
                        nc.scalar.activation(
                            W(S_FS), W(psl(l - 1, l - 1)), AF.Copy, scale=Em
                        )
                        nc.vector.tensor_tensor(
                            W(psl(l, l - 1)), W(S_FS), xh, OP.mult
                        )
                    if l >= 3:
                        nm = l - 2
                        for m in range(1, l - 1):
                            nc.scalar.activation(
                                W(S_XS + m - 1), xh, AF.Copy, scale=xa[(l, m)]
                            )
                        nc.vector.tensor_tensor(
                            WB(S_TM, nm), WB(S_XS, nm),
                            WB(psl(l - 1, 1), nm), OP.mult,
                        )
                        nc.vector.tensor_tensor(
                            WB(psl(l, 1), nm), WB(S_TM, nm),
                            WB(psl(l - 2, 1), nm), OP.subtract,
                        )
                    # emits: sin block then cos block (diagonal included)
                    cb = _lbase(l)
                    nc.vector.tensor_tensor(
                        O3[:, cb:cb + l, :], WB(psl(l, 1), l),
                        WB(S_SIN, l), OP.mult,
                    )
                    nc.vector.tensor_tensor(
                        O3[:, cb + l:cb + 2 * l, :], WB(psl(l, 1), l),
                        WB(S_COS, l), OP.mult,
                    )
                    while stripe_i < len(STRIPES) and STRIPES[stripe_i][2] == l:
                        c0, c1, _ = STRIPES[stripe_i]
                        qb = ooff * NCOL
                        nc.sync.dma_start(
                            out=outv[:, qb + c0 * fd:qb + c1 * fd],
                            in_=O[:, c0 * fd:c1 * fd],
                        )
                        stripe_i += 1
    _legalize_waits(nc)
    return nc


def _legalize_waits(nc):
    """TPB compute ISA structs encode a single sync-wait slot; Tile can
    emit 2+ waits on one instruction. Hoist extras onto NoOps."""
    f = nc.m.functions[0]
    for b in f.blocks:
        insts = b.instructions
        idx = 0
        while idx < len(insts):
            i = insts[idx]
            si = i.sync_info
            if si is not None and len(si.on_wait) > 1:
                waits = list(si.on_wait)
                for wextra in waits[:-1]:
                    nop = mybir.InstEventSemaphore(
                        name=nc.get_next_instruction_name(), ins=[], outs=[]
                    )
                    nop.engine = i.engine
                    nop.sync_info = mybir.SyncInfo(
                        on_wait=[wextra], on_update=[]
                    )
                    nc.register_instruction(nop)
                    insts.insert(idx, nop)
                    idx += 1
                si.on_wait = [waits[-1]]
            idx += 1


_NC_CACHE = None

# Smaller final chunk shrinks the exposed tail DMA.
FDS = [405, 405, 167]


def _get_nc():
    global _NC_CACHE
    if _NC_CACHE is None:
        _NC_CACHE = build_nc(FDS)
    return _NC_CACHE


def _run(cos_theta, phi, trace=False, **kw):
    cos_theta = np.ascontiguousarray(np.asarray(cos_theta), dtype=np.float32)
    phi = np.ascontiguousarray(np.asarray(phi), dtype=np.float32)
    assert cos_theta.shape == (N_TOTAL,) and phi.shape == (N_TOTAL,)
    in_maps = []
    for i in range(NCORES):
        c = np.zeros(PADN, np.float32)
        p_ = np.zeros(PADN, np.float32)
        c[:PER] = cos_theta[i * PER:(i + 1) * PER]
        p_[:PER] = phi[i * PER:(i + 1) * PER]
        in_maps.append({"cos_theta": c, "phi": p_})
    res = run_bass_kernel_spmd(
        _get_nc(), in_maps, core_ids=list(range(NCORES)), trace=trace, **kw
    )
    order, scale = _host_maps()
    outs = []
    for r in res.results:
        a = np.asarray(r["out"]).reshape(P, NCOL * LPP)
        parts, q = [], 0
        for fd in FDS:  # undo per-chunk framing -> [P, NCOL, LPP]
            parts.append(a[:, q:q + NCOL * fd].reshape(P, NCOL, fd))
            q += NCOL * fd
        dev = np.concatenate(parts, axis=2)
        dev = dev.transpose(0, 2, 1).reshape(PADN, NCOL)
        outs.append(dev[:PER, order].astype(np.float32) * scale[None, :])
    return np.concatenate(outs, axis=0), res


def kernel(cos_theta, phi):
    out, _ = _run(cos_theta, phi)
    return out
